# revision 12
# baseline (speedup 1.0000x reference)
"""Trainium2 Bass kernel for the embedding_lookup Classifier problem.

Computation (per token t):
    out[t] = relu(W1[:VOCAB][tk[t]] + hs0[t] @ W1[VOCAB:] + b1) @ W2 + b2

Sharding: data-parallel over the batch dim across 8 cores (2 batches =
8192 tokens per core); W1h / W2 / b2 replicated. The vocab-row gather
(a pure indexed copy) and the hs0 transpose are folded into host-side
shard prep: each core receives
  - hs0T   [768, 8192] f32r : its hs0 shard transposed (the contraction
           dim must land on SBUF partitions; fp32 DMA-transpose doesn't
           exist on TRN2),
  - tokT   [128, 8192] f32r : (W1[:VOCAB] + b1)[tk].T for its tokens,
  - w1h    [128, 768]  f32r : W1[VOCAB:] pre-chunked for lhsT loads,
  - ident  [128, 128]  f32r, w2 [128, 1] f32r, b2 [1, 1] f32.

Per-core device kernel ("h.T layout", hs1 on partitions):
  - PSUM bank [128 hs1, 512 tok] accumulates 7 f32r matmuls (full PE
    rate at moving free dim 512): identity.T @ tokT (the tok_part add)
    + 6 hidden-chunk projections.
  - relu on ACT -> SBUF (f32r), 128->1 contraction with W2 on PE,
    +b2 on DVE, small DMA out.
"""

import os

import numpy as np

HIDDEN = 768
VOCAB = 32000
HS1 = 128
B, S = 16, 4096
N_CORES = 8
T = (B // N_CORES) * S  # 8192 tokens per core
TB = 2048  # tokens per DMA block
SUB = 512  # tokens per PE sub-block (PSUM bank width in f32)
N_HC = HIDDEN // 128  # 6 hidden chunks

_CACHE = {}


def _build_nc():
    import concourse.bacc as bacc
    import concourse.mybir as mybir
    import concourse.tile as tile

    f32 = mybir.dt.float32
    f32r = mybir.dt.float32r
    RELU = mybir.ActivationFunctionType.Relu

    nc = bacc.Bacc("TRN2", debug=False, target_bir_lowering=False)

    hs0t = nc.dram_tensor("hs0t", [HIDDEN, T], f32r, kind="ExternalInput").ap()
    tokt = nc.dram_tensor("tokt", [128, T], f32r, kind="ExternalInput").ap()
    w1h = nc.dram_tensor("w1h", [128, HIDDEN], f32r, kind="ExternalInput").ap()
    ident = nc.dram_tensor("ident", [128, 128], f32r, kind="ExternalInput").ap()
    w2 = nc.dram_tensor("w2", [HS1, 1], f32r, kind="ExternalInput").ap()
    b2 = nc.dram_tensor("b2", [1, 1], f32, kind="ExternalInput").ap()
    out = nc.dram_tensor("out", [1, T], f32, kind="ExternalOutput").ap()

    with tile.TileContext(nc) as tc:
        with (
            tc.tile_pool(name="consts", bufs=1) as consts,
            tc.tile_pool(name="hs", bufs=2 * N_HC) as hs_pool,
            tc.tile_pool(name="tok", bufs=2) as tok_pool,
            tc.tile_pool(name="hrelu", bufs=3) as h_pool,
            tc.tile_pool(name="osb", bufs=4) as o_pool,
            tc.tile_pool(name="ps", bufs=2, space="PSUM") as psum_pool,
            tc.tile_pool(name="ps2", bufs=2, space="PSUM") as ps2_pool,
        ):
            w1h_sb = consts.tile([128, HIDDEN], f32r)
            nc.sync.dma_start(w1h_sb[:], w1h[:])
            id_sb = consts.tile([128, 128], f32r)
            nc.sync.dma_start(id_sb[:], ident[:])
            w2_sb = consts.tile([HS1, 1], f32r)
            nc.sync.dma_start(w2_sb[:], w2[:])
            b2_sb = consts.tile([1, 1], f32)
            nc.sync.dma_start(b2_sb[:], b2[:])

            hs0t_r = hs0t.rearrange("(c p) t -> p c t", p=128)

            deferred = []  # one-deep pipeline for the W2 dot + epilogue

            def epilogue(P, i):
                h = h_pool.tile([128, SUB], f32r, tag="h", name=f"h_{i}")
                nc.scalar.activation(h[:], P[:], RELU)
                P2 = ps2_pool.tile([1, SUB], f32, tag="P2", name=f"P2_{i}")
                nc.tensor.matmul(P2[:], w2_sb[:], h[:], start=True, stop=True)
                ot = o_pool.tile([1, SUB], f32, tag="ot", name=f"ot_{i}")
                nc.vector.tensor_scalar_add(ot[:], P2[:], b2_sb[:, :1])
                nc.sync.dma_start(out[:, i * SUB : (i + 1) * SUB], ot[:])

            for b in range(T // TB):
                hx = [
                    hs_pool.tile([128, TB], f32r, tag="hx", name=f"hx_{b}_{c}")
                    for c in range(N_HC)
                ]
                for c in range(N_HC):
                    nc.sync.dma_start(
                        hx[c][:], hs0t_r[:, c, b * TB : (b + 1) * TB]
                    )
                tokx = tok_pool.tile([128, TB], f32r, tag="tokx", name=f"tokx_{b}")
                nc.sync.dma_start(tokx[:], tokt[:, b * TB : (b + 1) * TB])
                for j in range(TB // SUB):
                    i = b * (TB // SUB) + j
                    ts = slice(j * SUB, (j + 1) * SUB)
                    P = psum_pool.tile([128, SUB], f32, tag="P", name=f"P_{i}")
                    # tok_part add first: identity.T @ tokT covers the
                    # full bank with start=True.
                    nc.tensor.matmul(
                        P[:], id_sb[:], tokx[:, ts], start=True, stop=False
                    )
                    for c in range(N_HC):
                        nc.tensor.matmul(
                            P[:],
                            w1h_sb[:, c * 128 : (c + 1) * 128],
                            hx[c][:, ts],
                            start=False,
                            stop=(c == N_HC - 1),
                        )
                    if deferred:
                        epilogue(*deferred.pop())
                    deferred.append((P, i))
            epilogue(*deferred.pop())

    nc.compile()
    return nc


def _prep_shared(W1, b1, W2, b2):
    W1 = np.asarray(W1, dtype=np.float32)
    b1 = np.asarray(b1, dtype=np.float32)
    w1tok = W1[:VOCAB] + b1[None, :]
    w1h = np.ascontiguousarray(
        W1[VOCAB:].reshape(N_HC, 128, HS1).transpose(1, 0, 2).reshape(128, N_HC * HS1)
    )
    ident = np.eye(128, dtype=np.float32)
    w2 = np.ascontiguousarray(np.asarray(W2, dtype=np.float32).reshape(HS1, 1))
    b2 = np.asarray(b2, dtype=np.float32).reshape(1, 1)
    return w1tok, w1h, ident, w2, b2


def _prep_core(tk, hs0, w1tok, c):
    nb = B // N_CORES
    tkc = np.asarray(tk[c * nb : (c + 1) * nb]).reshape(-1)
    tokt = np.ascontiguousarray(w1tok[tkc].T)  # [128, T]
    hs = np.asarray(hs0[c * nb : (c + 1) * nb], dtype=np.float32).reshape(T, HIDDEN)
    hs0t = np.ascontiguousarray(hs.T)
    return tokt, hs0t


def kernel(tk, hs0, W1, b1, W2, b2):
    from concourse.bass_utils import run_bass_kernel_spmd

    if "nc" not in _CACHE:
        _CACHE["nc"] = _build_nc()
    nc = _CACHE["nc"]

    w1tok, w1h, ident, w2, b2a = _prep_shared(W1, b1, W2, b2)
    in_maps = []
    for c in range(N_CORES):
        tokt, hs0t = _prep_core(tk, hs0, w1tok, c)
        in_maps.append(
            {
                "hs0t": hs0t,
                "tokt": tokt,
                "w1h": w1h,
                "ident": ident,
                "w2": w2,
                "b2": b2a,
            }
        )

    trace = bool(int(os.environ.get("KERNEL_TRACE", "0")))
    res = run_bass_kernel_spmd(
        nc, in_maps, core_ids=list(range(N_CORES)), trace=trace
    )
    _CACHE["last_results"] = res
    outs = [res.results[c]["out"].reshape(-1) for c in range(N_CORES)]
    return np.concatenate(outs).reshape(B, S).astype(np.float32)


# revision 14
# speedup vs baseline: 1.0660x; 1.0660x over previous
"""Trainium2 Bass kernel for the embedding_lookup Classifier problem.

Computation (per token t):
    out[t] = relu(W1[:VOCAB][tk[t]] + hs0[t] @ W1[VOCAB:] + b1) @ W2 + b2

Sharding: data-parallel over the batch dim across 8 cores (2 batches =
8192 tokens per core); W1h / W2 / b2 replicated. The vocab-row gather
(a pure indexed copy) and the hs0 transpose are folded into host-side
shard prep: each core receives
  - hs0T   [768, 8192] f32r : its hs0 shard transposed (the contraction
           dim must land on SBUF partitions; fp32 DMA-transpose doesn't
           exist on TRN2),
  - tokT   [128, 8192] f32r : (W1[:VOCAB] + b1)[tk].T for its tokens,
  - w1h    [128, 768]  f32r : W1[VOCAB:] pre-chunked for lhsT loads,
  - ident  [128, 128]  f32r, w2 [128, 1] f32r, b2 [1, 1] f32.

Per-core device kernel ("h.T layout", hs1 on partitions):
  - PSUM bank [128 hs1, 512 tok] accumulates 7 f32r matmuls (full PE
    rate at moving free dim 512): identity.T @ tokT (the tok_part add)
    + 6 hidden-chunk projections.
  - relu on ACT -> SBUF (f32r), 128->1 contraction with W2 on PE,
    +b2 on DVE, small DMA out.
"""

import os

import numpy as np

HIDDEN = 768
VOCAB = 32000
HS1 = 128
B, S = 16, 4096
N_CORES = 8
T = (B // N_CORES) * S  # 8192 tokens per core
TB = 1024  # tokens per DMA block
SUB = 512  # tokens per PE sub-block (PSUM bank width in f32)
N_HC = HIDDEN // 128  # 6 hidden chunks

_CACHE = {}


def _build_nc():
    import concourse.bacc as bacc
    import concourse.mybir as mybir
    import concourse.tile as tile

    f32 = mybir.dt.float32
    f32r = mybir.dt.float32r
    RELU = mybir.ActivationFunctionType.Relu

    nc = bacc.Bacc("TRN2", debug=False, target_bir_lowering=False)

    hs0t = nc.dram_tensor("hs0t", [HIDDEN, T], f32r, kind="ExternalInput").ap()
    tokt = nc.dram_tensor("tokt", [128, T], f32r, kind="ExternalInput").ap()
    w1h = nc.dram_tensor("w1h", [128, HIDDEN], f32r, kind="ExternalInput").ap()
    ident = nc.dram_tensor("ident", [128, 128], f32r, kind="ExternalInput").ap()
    w2 = nc.dram_tensor("w2", [HS1, 1], f32r, kind="ExternalInput").ap()
    b2 = nc.dram_tensor("b2", [1, 1], f32, kind="ExternalInput").ap()
    out = nc.dram_tensor("out", [1, T], f32, kind="ExternalOutput").ap()

    with tile.TileContext(nc) as tc:
        with (
            tc.tile_pool(name="consts", bufs=1) as consts,
            tc.tile_pool(name="hs", bufs=3) as hs_pool,
            tc.tile_pool(name="tok", bufs=3) as tok_pool,
            tc.tile_pool(name="hrelu", bufs=3) as h_pool,
            tc.tile_pool(name="osb", bufs=4) as o_pool,
            tc.tile_pool(name="ps", bufs=2, space="PSUM") as psum_pool,
            tc.tile_pool(name="ps2", bufs=2, space="PSUM") as ps2_pool,
        ):
            w1h_sb = consts.tile([128, HIDDEN], f32r)
            nc.sync.dma_start(w1h_sb[:], w1h[:])
            id_sb = consts.tile([128, 128], f32r)
            nc.sync.dma_start(id_sb[:], ident[:])
            w2_sb = consts.tile([HS1, 1], f32r)
            nc.sync.dma_start(w2_sb[:], w2[:])
            b2_sb = consts.tile([1, 1], f32)
            nc.sync.dma_start(b2_sb[:], b2[:])

            hs0t_r = hs0t.rearrange("(c p) t -> p c t", p=128)

            deferred = []  # one-deep pipeline for the W2 dot + epilogue

            def epilogue(P, i):
                h = h_pool.tile([128, SUB], f32r, tag="h", name=f"h_{i}")
                nc.scalar.activation(h[:], P[:], RELU)
                P2 = ps2_pool.tile([1, SUB], f32, tag="P2", name=f"P2_{i}")
                nc.tensor.matmul(P2[:], w2_sb[:], h[:], start=True, stop=True)
                ot = o_pool.tile([1, SUB], f32, tag="ot", name=f"ot_{i}")
                nc.vector.tensor_scalar_add(ot[:], P2[:], b2_sb[:, :1])
                nc.sync.dma_start(out[:, i * SUB : (i + 1) * SUB], ot[:])

            for b in range(T // TB):
                hxt = hs_pool.tile(
                    [128, N_HC, TB], f32r, tag="hx", name=f"hx_{b}"
                )
                nc.sync.dma_start(hxt[:], hs0t_r[:, :, b * TB : (b + 1) * TB])
                hx = [hxt[:, c, :] for c in range(N_HC)]
                tokx = tok_pool.tile([128, TB], f32r, tag="tokx", name=f"tokx_{b}")
                nc.sync.dma_start(tokx[:], tokt[:, b * TB : (b + 1) * TB])
                for j in range(TB // SUB):
                    i = b * (TB // SUB) + j
                    ts = slice(j * SUB, (j + 1) * SUB)
                    P = psum_pool.tile([128, SUB], f32, tag="P", name=f"P_{i}")
                    # tok_part add first: identity.T @ tokT covers the
                    # full bank with start=True.
                    nc.tensor.matmul(
                        P[:], id_sb[:], tokx[:, ts], start=True, stop=False
                    )
                    for c in range(N_HC):
                        nc.tensor.matmul(
                            P[:],
                            w1h_sb[:, c * 128 : (c + 1) * 128],
                            hx[c][:, ts],
                            start=False,
                            stop=(c == N_HC - 1),
                        )
                    if deferred:
                        epilogue(*deferred.pop())
                    deferred.append((P, i))
            epilogue(*deferred.pop())

    nc.compile()
    return nc


def _prep_shared(W1, b1, W2, b2):
    W1 = np.asarray(W1, dtype=np.float32)
    b1 = np.asarray(b1, dtype=np.float32)
    w1tok = W1[:VOCAB] + b1[None, :]
    w1h = np.ascontiguousarray(
        W1[VOCAB:].reshape(N_HC, 128, HS1).transpose(1, 0, 2).reshape(128, N_HC * HS1)
    )
    ident = np.eye(128, dtype=np.float32)
    w2 = np.ascontiguousarray(np.asarray(W2, dtype=np.float32).reshape(HS1, 1))
    b2 = np.asarray(b2, dtype=np.float32).reshape(1, 1)
    return w1tok, w1h, ident, w2, b2


def _prep_core(tk, hs0, w1tok, c):
    nb = B // N_CORES
    tkc = np.asarray(tk[c * nb : (c + 1) * nb]).reshape(-1)
    tokt = np.ascontiguousarray(w1tok[tkc].T)  # [128, T]
    hs = np.asarray(hs0[c * nb : (c + 1) * nb], dtype=np.float32).reshape(T, HIDDEN)
    hs0t = np.ascontiguousarray(hs.T)
    return tokt, hs0t


def kernel(tk, hs0, W1, b1, W2, b2):
    from concourse.bass_utils import run_bass_kernel_spmd

    if "nc" not in _CACHE:
        _CACHE["nc"] = _build_nc()
    nc = _CACHE["nc"]

    w1tok, w1h, ident, w2, b2a = _prep_shared(W1, b1, W2, b2)
    in_maps = []
    for c in range(N_CORES):
        tokt, hs0t = _prep_core(tk, hs0, w1tok, c)
        in_maps.append(
            {
                "hs0t": hs0t,
                "tokt": tokt,
                "w1h": w1h,
                "ident": ident,
                "w2": w2,
                "b2": b2a,
            }
        )

    trace = bool(int(os.environ.get("KERNEL_TRACE", "0")))
    res = run_bass_kernel_spmd(
        nc, in_maps, core_ids=list(range(N_CORES)), trace=trace
    )
    _CACHE["last_results"] = res
    outs = [res.results[c]["out"].reshape(-1) for c in range(N_CORES)]
    return np.concatenate(outs).reshape(B, S).astype(np.float32)


# revision 15
# speedup vs baseline: 1.1210x; 1.0517x over previous
"""Trainium2 Bass kernel for the embedding_lookup Classifier problem.

Computation (per token t):
    out[t] = relu(W1[:VOCAB][tk[t]] + hs0[t] @ W1[VOCAB:] + b1) @ W2 + b2

Sharding: data-parallel over the batch dim across 8 cores (2 batches =
8192 tokens per core); W1h / W2 / b2 replicated. The vocab-row gather
(a pure indexed copy) and the hs0 transpose are folded into host-side
shard prep: each core receives
  - hsx [896, 8192] f32r : rows 0..767 = the hs0 shard transposed (the
        contraction dim must land on SBUF partitions; fp32 DMA-transpose
        doesn't exist on TRN2), rows 768..895 = (W1[:VOCAB]+b1)[tk].T
        (the vocab gather is a pure indexed copy -> shard prep),
  - w1x [128, 896] f32r : W1[VOCAB:] pre-chunked for lhsT loads, plus
        identity as chunk 6 (adds tok_part into the accumulation),
  - w2 [128, 1] f32r, b2 [1, 1] f32.

Per-core device kernel ("h.T layout", hs1 on partitions):
  - PSUM bank [128 hs1, 512 tok] accumulates 7 uniform f32r matmuls
    (full PE rate at moving free dim 512) contracting 896 rows.
  - relu on ACT -> SBUF (f32r), 128->1 contraction with W2 on PE,
    +b2 on DVE, small DMA out.
"""

import os

import numpy as np

HIDDEN = 768
VOCAB = 32000
HS1 = 128
B, S = 16, 4096
N_CORES = 8
T = (B // N_CORES) * S  # 8192 tokens per core
TB = 512  # tokens per DMA block
SUB = 512  # tokens per PE sub-block (PSUM bank width in f32)
N_HC = HIDDEN // 128  # 6 hidden chunks
N_C = N_HC + 1  # + tok chunk

_CACHE = {}


def _build_nc():
    import concourse.bacc as bacc
    import concourse.mybir as mybir
    import concourse.tile as tile

    f32 = mybir.dt.float32
    f32r = mybir.dt.float32r
    RELU = mybir.ActivationFunctionType.Relu

    nc = bacc.Bacc("TRN2", debug=False, target_bir_lowering=False)

    hsx = nc.dram_tensor("hsx", [N_C * 128, T], f32r, kind="ExternalInput").ap()
    w1x = nc.dram_tensor("w1x", [128, N_C * 128], f32r, kind="ExternalInput").ap()
    w2 = nc.dram_tensor("w2", [HS1, 1], f32r, kind="ExternalInput").ap()
    b2 = nc.dram_tensor("b2", [1, 1], f32, kind="ExternalInput").ap()
    out = nc.dram_tensor("out", [1, T], f32, kind="ExternalOutput").ap()

    with tile.TileContext(nc) as tc:
        with (
            tc.tile_pool(name="consts", bufs=1) as consts,
            tc.tile_pool(name="hs", bufs=6) as hs_pool,
            tc.tile_pool(name="hrelu", bufs=3) as h_pool,
            tc.tile_pool(name="osb", bufs=4) as o_pool,
            tc.tile_pool(name="ps", bufs=2, space="PSUM") as psum_pool,
            tc.tile_pool(name="ps2", bufs=2, space="PSUM") as ps2_pool,
        ):
            hsx_r = hsx.rearrange("(c p) t -> p c t", p=128)
            NB = T // TB

            # issue the first input blocks before the tiny const loads so
            # the big DMA stream starts as early as possible
            hxts = []
            def load_block(b):
                hxt = hs_pool.tile([128, N_C, TB], f32r, tag="hx", name=f"hx_{b}")
                nc.sync.dma_start(hxt[:], hsx_r[:, :, b * TB : (b + 1) * TB])
                hxts.append(hxt)
            load_block(0)
            load_block(1)

            w1x_sb = consts.tile([128, N_C * 128], f32r)
            nc.sync.dma_start(w1x_sb[:], w1x[:])
            w2_sb = consts.tile([HS1, 1], f32r)
            nc.sync.dma_start(w2_sb[:], w2[:])
            b2_sb = consts.tile([1, 1], f32)
            nc.sync.dma_start(b2_sb[:], b2[:])

            deferred = []  # one-deep pipeline for the W2 dot + epilogue

            def epilogue(P, i):
                h = h_pool.tile([128, SUB], f32r, tag="h", name=f"h_{i}")
                nc.scalar.activation(h[:], P[:], RELU)
                P2 = ps2_pool.tile([1, SUB], f32, tag="P2", name=f"P2_{i}")
                nc.tensor.matmul(P2[:], w2_sb[:], h[:], start=True, stop=True)
                ot = o_pool.tile([1, SUB], f32, tag="ot", name=f"ot_{i}")
                nc.vector.tensor_scalar_add(ot[:], P2[:], b2_sb[:, :1])
                nc.sync.dma_start(out[:, i * SUB : (i + 1) * SUB], ot[:])

            for b in range(NB):
                if b + 2 < NB:
                    load_block(b + 2)
                hxt = hxts[b]
                for j in range(TB // SUB):
                    i = b * (TB // SUB) + j
                    ts = slice(j * SUB, (j + 1) * SUB)
                    P = psum_pool.tile([128, SUB], f32, tag="P", name=f"P_{i}")
                    for c in range(N_C):
                        nc.tensor.matmul(
                            P[:],
                            w1x_sb[:, c * 128 : (c + 1) * 128],
                            hxt[:, c, ts],
                            start=(c == 0),
                            stop=(c == N_C - 1),
                        )
                    if deferred:
                        epilogue(*deferred.pop())
                    deferred.append((P, i))
            epilogue(*deferred.pop())

    nc.compile()
    return nc


def _prep_shared(W1, b1, W2, b2):
    W1 = np.asarray(W1, dtype=np.float32)
    b1 = np.asarray(b1, dtype=np.float32)
    w1tok = W1[:VOCAB] + b1[None, :]
    w1h = W1[VOCAB:].reshape(N_HC, 128, HS1).transpose(1, 0, 2).reshape(128, N_HC * HS1)
    w1x = np.ascontiguousarray(
        np.concatenate([w1h, np.eye(128, dtype=np.float32)], axis=1)
    )
    w2 = np.ascontiguousarray(np.asarray(W2, dtype=np.float32).reshape(HS1, 1))
    b2 = np.asarray(b2, dtype=np.float32).reshape(1, 1)
    return w1tok, w1x, w2, b2


def _prep_core(tk, hs0, w1tok, c):
    nb = B // N_CORES
    tkc = np.asarray(tk[c * nb : (c + 1) * nb]).reshape(-1)
    hs = np.asarray(hs0[c * nb : (c + 1) * nb], dtype=np.float32).reshape(T, HIDDEN)
    hsx = np.empty((N_C * 128, T), dtype=np.float32)
    hsx[:HIDDEN] = hs.T
    hsx[HIDDEN:] = w1tok[tkc].T
    return hsx


def kernel(tk, hs0, W1, b1, W2, b2):
    from concourse.bass_utils import run_bass_kernel_spmd

    if "nc" not in _CACHE:
        _CACHE["nc"] = _build_nc()
    nc = _CACHE["nc"]

    w1tok, w1x, w2, b2a = _prep_shared(W1, b1, W2, b2)
    in_maps = []
    for c in range(N_CORES):
        hsx = _prep_core(tk, hs0, w1tok, c)
        in_maps.append({"hsx": hsx, "w1x": w1x, "w2": w2, "b2": b2a})

    trace = bool(int(os.environ.get("KERNEL_TRACE", "0")))
    res = run_bass_kernel_spmd(
        nc, in_maps, core_ids=list(range(N_CORES)), trace=trace
    )
    _CACHE["last_results"] = res
    outs = [res.results[c]["out"].reshape(-1) for c in range(N_CORES)]
    return np.concatenate(outs).reshape(B, S).astype(np.float32)
